# revision 6
# baseline (speedup 1.0000x reference)
"""Dynamic-kernel CNN (conv5x5 -> tanh gate -> windowed sum) on 8 trn2 cores.

out(y,x) = sum_t V_t(y,x) * tanh( sum_k W2[t,k] V_k(y,x) + b_t ),
t = k = (row-shift a, col-shift b); V_t = the 28x28 window of pad4(x) at
offset (2+a, 2+b)  (dense im2col, 25 taps per image).

Data-parallel over batch: 2048 images -> 256 per core (padded to 270 =
6 units of 9 five-image groups).

The kernel is a single pixel stream: 52 groups x 784 px = 40768 px,
processed in 35 half-slots of 1176 px (= 3 PSUM banks at 392 px each).
The tanh ACTIVATE is the binding engine (~1.24us per slot at the
measured 1.24 GHz ACT rate), so everything else is arranged to keep it
back-to-back:
  - FC matmuls for slot H+2 ping-pong into the OTHER 3 banks of one
    persistent 6-bank PSUM ring while ACT reads slot H (subtile deps).
  - The channel-reduce matmuls are emitted TWO SLOTS behind their mul,
    so no PE instruction ahead of an FC ever blocks on fresh ACT/DVE
    output (v3 lost ~40% to exactly that: reduce waiting on mul in
    front of the next FC in the PE FIFO).
  - All gathers + consts ride SWDGE (gpsimd, 16 DMA engines); HWDGE
    (sync) only carries small y stores.  Gathers move one 1.76 MB unit
    (45 images, im2col'd on the host in unit order so the flat element
    streams match exactly); 14 KB per-partition descriptors amortize
    the ~300ns/descriptor cost that throttled v3's 4.7KB gathers.
    Unit 0 is split 1/2/3/3 groups so the first FC starts ~1.5us after
    the preamble instead of ~14us.

Per-core layout: partitions q = 25*s + t (s = image-in-group, t = tap),
free dim = pixel stream.

Pipeline per half-slot H (1176 px):
  1. 3 FC matmuls (N=392) into PSUM banks 3*(H%2)+i
  2. one strided ACTIVATE [125, 3, 392] -> G (bf16, bias fused)
  3. M = V*G on DVE (bf16, 2x)
  4. lag-2: shifted-ones reduce matmuls for every group newly complete
     at slot H-2, accumulating rounds (25 groups) into a 2-bank PSUM
     tile; on round end: DVE-evac -> y-store (r0,r2 on sync, r1 on
     gpsimd so the tail drains both rings in parallel).
"""

import numpy as np
from contextlib import ExitStack

import concourse.bass as bass
import concourse.tile as tile
from concourse import bacc, mybir
from concourse import bass_utils

F32 = mybir.dt.float32
BF16 = mybir.dt.bfloat16
TANH = mybir.ActivationFunctionType.Tanh

N_CORES = 8
B_FULL = 2048
B_LOC = B_FULL // N_CORES   # 256
NPIX = 784                  # 28*28
VROW = 25 * NPIX            # 19600 im2col elements per image
N_UNITS = 6                 # 6 units x 45 images = 270 slots (256 real)
UPIX = 9 * NPIX             # 7056 px per unit = 6 half-slots
HPIX = 1176                 # px per half-slot (3 banks x 392)
N_GROUPS = 52               # real 5-image groups (images 0..259)
N_HALF = 35                 # ceil(52*784 / 1176)

# group g -> (round r, j): round r stores y rows [125r : 125r+rows]
def _round_of(g):
    r = 2 if g >= 50 else g // 25
    return r, g - 25 * r

ROUND_ROWS = (125, 125, 6)
ROUND_NGROUPS = (25, 25, 2)


def _emit(ctx, tc, v_d, wc_d, bias_d, y_d):
    nc = tc.nc

    cpool = ctx.enter_context(tc.tile_pool(name="const", bufs=1))
    vpool = ctx.enter_context(tc.tile_pool(name="v", bufs=3))
    gpool = ctx.enter_context(tc.tile_pool(name="g", bufs=2))
    mpool = ctx.enter_context(tc.tile_pool(name="m", bufs=3))
    epool = ctx.enter_context(tc.tile_pool(name="evac", bufs=2))
    pfc = ctx.enter_context(tc.tile_pool(name="pfc", bufs=1, space="PSUM"))
    pred = ctx.enter_context(tc.tile_pool(name="pred", bufs=1, space="PSUM"))

    # consts + gathers all ride SWDGE (gpsimd, 16 DMA engines)
    biasv = cpool.tile([125, 1], F32)
    nc.gpsimd.dma_start(biasv[:], bias_d[:])
    wconsts = cpool.tile([125, 370], BF16)   # wblk | mbig packed
    nc.gpsimd.dma_start(wconsts[:], wc_d[:])
    wblk = wconsts[:, 0:125]

    # one persistent 6-bank FC ring: half-slot H uses banks 3*(H%2)+i
    fc_ring = pfc.tile([125, 3072], F32)
    fc_banks = fc_ring[:].rearrange("p (t c) -> p t c", c=512)

    red_tiles = {}
    m_info = [None] * N_GROUPS   # g -> (m_tile, col offset)
    state = {"next_red": 0}

    def emit_reduce_upto(n):
        while state["next_red"] < min(n, N_GROUPS):
            g = state["next_red"]
            state["next_red"] += 1
            m, off = m_info[g]
            r, j = _round_of(g)
            if j == 0:
                red_tiles[r] = pred.tile([125, 1024], F32,
                                         name="red", tag="red")
            red = red_tiles[r]
            ones_j = wconsts[:, 245 - 5 * j : 370 - 5 * j]
            last = j == ROUND_NGROUPS[r] - 1
            nc.tensor.matmul(
                red[:, 0:512], ones_j, m[:, off : off + 512],
                start=(j == 0), stop=last, skip_group_check=True,
            )
            nc.tensor.matmul(
                red[:, 512:784], ones_j, m[:, off + 512 : off + 784],
                start=(j == 0), stop=last, skip_group_check=True,
            )
            m_info[g] = None
            if last:
                rows = ROUND_ROWS[r]
                e = epool.tile([125, NPIX], F32, tag="evac")
                nc.vector.tensor_copy(e[:], red[:, 0:NPIX])
                # r1 completes at the very tail: ride gpsimd's 16
                # engines while sync drains r2's small store in parallel
                eng = nc.gpsimd if r == 1 else nc.sync
                eng.dma_start(y_d[125 * r : 125 * r + rows, :], e[0:rows, :])
                del red_tiles[r]

    H = 0
    for u in range(N_UNITS):
        # --- 1. gather this unit's im2col windows (SWDGE) ---
        v = vpool.tile([125, UPIX], BF16)
        if u == 0:
            # ramp: split 1/2/3/3 groups so FC can start ~1.5us in
            src = v_d[0:1, :].rearrange("o (s t x) -> (o s) t x", s=5, t=25)
            for a, b in ((0, 1), (1, 3), (3, 6), (6, 9)):
                nc.gpsimd.dma_start(
                    v[:, 784 * a : 784 * b], src[:, :, 784 * a : 784 * b]
                )
        else:
            nc.gpsimd.dma_start(v[:], v_d[u : u + 1, :])

        g_t = gpool.tile([125, UPIX], BF16)
        m = mpool.tile([125, UPIX], BF16)
        for k in range(9):
            g = 9 * u + k
            if g < N_GROUPS:
                m_info[g] = (m, 784 * k)

        n_half = 5 if u == N_UNITS - 1 else 6
        for h in range(n_half):
            p = H % 2
            # --- 2. FC matmuls into banks 3p..3p+2 ---
            for i in range(3):
                col = HPIX * h + 392 * i
                nc.tensor.matmul(
                    fc_banks[:, 3 * p + i, 0:392], wblk,
                    v[:, col : col + 392],
                    start=True, stop=True,
                )
            # --- 3. G = tanh(FC + b), one strided ACT ---
            fcv = fc_banks[:, 3 * p : 3 * p + 3, 0:392]
            gv = g_t[:, HPIX * h : HPIX * (h + 1)].rearrange(
                "p (t c) -> p t c", c=392
            )
            nc.scalar.activation(gv, fcv, TANH, bias=biasv[:], scale=1.0)
            # --- 4. M = V * G (DVE) ---
            nc.vector.tensor_mul(
                m[:, HPIX * h : HPIX * (h + 1)],
                v[:, HPIX * h : HPIX * (h + 1)],
                g_t[:, HPIX * h : HPIX * (h + 1)],
            )
            H += 1
            # --- 5. reduce, two half-slots behind ---
            if H >= 3:
                emit_reduce_upto((3 * (H - 2)) // 2)

    emit_reduce_upto(N_GROUPS)


def build():
    nc = bacc.Bacc("TRN2", target_bir_lowering=False, debug=False)
    v_d = nc.dram_tensor("vwin", [N_UNITS, 45 * VROW], BF16,
                         kind="ExternalInput").ap()
    wc_d = nc.dram_tensor("wconsts", [125, 370], BF16,
                          kind="ExternalInput").ap()
    bias_d = nc.dram_tensor("biasv", [125, 1], F32, kind="ExternalInput").ap()
    y_d = nc.dram_tensor("y", [B_LOC, NPIX], F32, kind="ExternalOutput").ap()

    with tile.TileContext(nc) as tc:
        with ExitStack() as ctx:
            _emit(ctx, tc, v_d, wc_d, bias_d, y_d)
    nc.compile()
    return nc


def make_consts(W, b):
    import ml_dtypes

    W = np.asarray(W, dtype=np.float32)
    b = np.asarray(b, dtype=np.float32)
    # tap index t = 5a+bb (a=row-shift, bb=col-shift); gate channel at
    # slot q=(aq,bq) is conv output channel c = 5*bq+aq
    perm = np.array([5 * (q % 5) + q // 5 for q in range(25)])
    W2t = W[:, 0].reshape(25, 25)          # W2t[c, t] = W[c,0,a,bb]
    wsmall = W2t[perm].T                   # wsmall[t, q] = W2t[perm[q], t]
    wc = np.zeros((125, 370), dtype=np.float32)
    for s in range(5):
        wc[25 * s : 25 * s + 25, 25 * s : 25 * s + 25] = wsmall  # wblk
        wc[25 * s : 25 * s + 25, 125 + 120 + s] = 1.0            # mbig
    biasv = np.tile(b[perm], 5).astype(np.float32)[:, None]
    return wc.astype(ml_dtypes.bfloat16), biasv


def make_windows(x):
    """Dense im2col in unit order: vwin[core][u, s, t, g', pix] =
    pad4(x)[img=45u+5g'+s, 2+a+y, 2+bb+x], bf16; image slots >= 256 zero."""
    import ml_dtypes

    x = np.asarray(x, dtype=np.float32).reshape(B_FULL, 28, 28)
    xp4 = np.pad(x, ((0, 0), (4, 4), (4, 4)))
    win = np.lib.stride_tricks.sliding_window_view(xp4, (28, 28), axis=(1, 2))
    win = win[:, 2:7, 2:7]                 # [B, 5, 5, 28, 28]
    win = win.reshape(N_CORES, B_LOC, 25, NPIX).astype(ml_dtypes.bfloat16)
    vw = np.zeros((N_CORES, 45 * N_UNITS, 25, NPIX),
                  dtype=ml_dtypes.bfloat16)
    vw[:, :B_LOC] = win
    # [c, (u g' s), t, pix] -> [c, u, s, t, g', pix]
    vw = vw.reshape(N_CORES, N_UNITS, 9, 5, 25, NPIX)
    vw = vw.transpose(0, 1, 3, 4, 2, 5)
    return np.ascontiguousarray(
        vw.reshape(N_CORES, N_UNITS, 45 * VROW)
    )


_NC_CACHE = None


def get_nc():
    global _NC_CACHE
    if _NC_CACHE is None:
        _NC_CACHE = build()
    return _NC_CACHE


def run(x, W, b, **spmd_kwargs):
    wc, biasv = make_consts(W, b)
    vw = make_windows(x)
    in_maps = [
        {"vwin": vw[c], "wconsts": wc, "biasv": biasv}
        for c in range(N_CORES)
    ]
    nc = get_nc()
    res = bass_utils.run_bass_kernel_spmd(
        nc, in_maps, list(range(N_CORES)), **spmd_kwargs
    )
    y = np.concatenate([res.results[c]["y"] for c in range(N_CORES)], axis=0)
    return y.reshape(B_FULL, 1, 28, 28), res


def kernel(x, W, b):
    y, _ = run(x, W, b)
    return y.astype(np.float32)


# revision 7
# speedup vs baseline: 1.2281x; 1.2281x over previous
"""Dynamic-kernel CNN (conv5x5 -> tanh gate -> windowed sum) on 8 trn2 cores.

out(y,x) = sum_t V_t(y,x) * tanh( sum_k W2[t,k] V_k(y,x) + b_t ),
t = k = (row-shift a, col-shift b); V_t = the 28x28 window of pad4(x) at
offset (2+a, 2+b)  (dense im2col, 25 taps per image).

Data-parallel over batch: 2048 images -> 256 per core (padded to 270 =
6 units of 9 five-image groups).

The kernel is a single pixel stream: 52 groups x 784 px = 40768 px,
processed in 35 half-slots of 1176 px (3 PSUM banks x 392).  The tanh
ACTIVATE (~1.24us/slot at the measured 1.24 GHz ACT rate) is the
binding engine; v3/v4 lost ~0.8us/slot because ACT's G-tile write
carried a (conservatively tracked) WAR on the previous slot's DVE mul,
serializing ACT -> mul -> ACT.  v5 gives every half-slot its own G tile
and FC PSUM tile (pool bufs cycle them), so the only runtime edges per
ACT are "FC done" (resolved during the previous ACT) and 4-slot-old
tile reuse.  The channel reduce is emitted a full unit behind its muls.

Gathers: host ships dense im2col windows in unit order
(vwin[u, s, t, g', pix], one 1.76 MB row per unit) and each unit is
fetched by TWO DMAs split across the rings -- HWDGE/sync (5 SDMA
engines, ~90 GB/s) takes 3 groups, SWDGE/gpsimd (16 engines,
~150-200 GB/s) takes 6 -- because a single ring cannot sustain the
224 GB/s the back-to-back ACT cadence demands.  14 KB per-partition
descriptors amortize the ~300ns/descriptor fixed cost.  Unit 0 is
split 1/2/6 so the first FC starts a few us after the preamble.

Per-core layout: partitions q = 25*s + t (s = image-in-group, t = tap),
free dim = pixel stream.  Consts (wblk | shifted-ones | bias) ride one
packed bf16 tensor; bf16 bias costs < 1e-3 relative error.

Pipeline per half-slot H:
  1. 3 FC matmuls (N=392) into this slot's 3-bank PSUM tile
  2. one strided ACTIVATE [125, 3, 392] -> G_H (bf16, bias fused)
  3. M = V*G on DVE (bf16, 2x)
  4. unit-lagged shifted-ones reduce matmuls (group j -> partitions
     5j..5j+4), accumulating rounds (25 groups) into a 2-bank PSUM
     tile; round end: DVE-evac -> y-store (r0,r2 sync / r1 gpsimd).
"""

import numpy as np
from contextlib import ExitStack

import concourse.bass as bass
import concourse.tile as tile
from concourse import bacc, mybir
from concourse import bass_utils

F32 = mybir.dt.float32
BF16 = mybir.dt.bfloat16
TANH = mybir.ActivationFunctionType.Tanh

N_CORES = 8
B_FULL = 2048
B_LOC = B_FULL // N_CORES   # 256
NPIX = 784                  # 28*28
VROW = 25 * NPIX            # 19600 im2col elements per image
N_UNITS = 6                 # 6 units x 45 images = 270 slots (256 real)
UPIX = 9 * NPIX             # 7056 px per unit = 6 half-slots
HPIX = 1176                 # px per half-slot (3 banks x 392)
N_GROUPS = 52               # real 5-image groups (images 0..259)

# reduce emission: during unit u, after local slot h, groups of unit u-1
# up to 9*(u-1) + RED_CUM[h] are emitted (spread 9 over 6 slots)
RED_CUM = (1, 3, 4, 6, 7, 9)

def _round_of(g):
    r = 2 if g >= 50 else g // 25
    return r, g - 25 * r

ROUND_ROWS = (125, 125, 6)
ROUND_NGROUPS = (25, 25, 2)


def _emit(ctx, tc, v_d, wc_d, y_d):
    nc = tc.nc

    cpool = ctx.enter_context(tc.tile_pool(name="const", bufs=1))
    vpool = ctx.enter_context(tc.tile_pool(name="v", bufs=4))
    gpool = ctx.enter_context(tc.tile_pool(name="g", bufs=4))
    mpool = ctx.enter_context(tc.tile_pool(name="m", bufs=3))
    epool = ctx.enter_context(tc.tile_pool(name="evac", bufs=2))
    pfc = ctx.enter_context(tc.tile_pool(name="pfc", bufs=2, space="PSUM"))
    pred = ctx.enter_context(tc.tile_pool(name="pred", bufs=1, space="PSUM"))

    # packed consts: wblk | shifted-ones | bias (bf16), one gpsimd DMA
    wconsts = cpool.tile([125, 372], BF16)
    nc.gpsimd.dma_start(wconsts[:], wc_d[:])
    wblk = wconsts[:, 0:125]
    biasv = wconsts[:, 370:371]

    red_tiles = {}
    m_info = [None] * N_GROUPS   # g -> (m_tile, col offset)
    state = {"next_red": 0}

    def emit_reduce_upto(n):
        while state["next_red"] < min(n, N_GROUPS):
            g = state["next_red"]
            state["next_red"] += 1
            m, off = m_info[g]
            r, j = _round_of(g)
            if j == 0:
                red_tiles[r] = pred.tile([125, 1024], F32,
                                         name="red", tag="red")
            red = red_tiles[r]
            ones_j = wconsts[:, 245 - 5 * j : 370 - 5 * j]
            last = j == ROUND_NGROUPS[r] - 1
            nc.tensor.matmul(
                red[:, 0:512], ones_j, m[:, off : off + 512],
                start=(j == 0), stop=last, skip_group_check=True,
            )
            nc.tensor.matmul(
                red[:, 512:784], ones_j, m[:, off + 512 : off + 784],
                start=(j == 0), stop=last, skip_group_check=True,
            )
            m_info[g] = None
            if last:
                rows = ROUND_ROWS[r]
                e = epool.tile([125, NPIX], F32, tag="evac")
                nc.vector.tensor_copy(e[:], red[:, 0:NPIX])
                # r1 completes at the very tail: ride gpsimd's 16
                # engines while sync drains r2's small store in parallel
                eng = nc.gpsimd if r == 1 else nc.sync
                eng.dma_start(y_d[125 * r : 125 * r + rows, :], e[0:rows, :])
                del red_tiles[r]

    for u in range(N_UNITS):
        # --- 1. gather this unit's im2col windows, split across rings ---
        v = vpool.tile([125, UPIX], BF16)
        src = v_d[u : u + 1, :].rearrange("o (s t x) -> (o s) t x",
                                          s=5, t=25)
        if u == 0:
            parts = (((0, 1), nc.gpsimd), ((1, 3), nc.sync),
                     ((3, 9), nc.gpsimd))
        else:
            parts = (((0, 3), nc.sync), ((3, 9), nc.gpsimd))
        for (a, b), eng in parts:
            eng.dma_start(
                v[:, 784 * a : 784 * b], src[:, :, 784 * a : 784 * b]
            )

        m = mpool.tile([125, UPIX], BF16)
        for k in range(9):
            g = 9 * u + k
            if g < N_GROUPS:
                m_info[g] = (m, 784 * k)

        n_half = 5 if u == N_UNITS - 1 else 6
        for h in range(n_half):
            # --- 2. FC matmuls into this slot's own 3-bank PSUM tile ---
            fc = pfc.tile([125, 1536], F32, tag="fc")
            fcb = fc.rearrange("p (t c) -> p t c", c=512)
            for i in range(3):
                col = HPIX * h + 392 * i
                nc.tensor.matmul(
                    fcb[:, i, 0:392], wblk, v[:, col : col + 392],
                    start=True, stop=True,
                )
            # --- 3. G = tanh(FC + b), one strided ACT, own G tile ---
            g_t = gpool.tile([125, HPIX], BF16, tag="g")
            gv = g_t[:].rearrange("p (t c) -> p t c", c=392)
            nc.scalar.activation(gv, fcb[:, :, 0:392], TANH,
                                 bias=biasv, scale=1.0)
            # --- 4. M = V * G (DVE) ---
            nc.vector.tensor_mul(
                m[:, HPIX * h : HPIX * (h + 1)],
                v[:, HPIX * h : HPIX * (h + 1)],
                g_t[:],
            )
            # --- 5. reduce, one unit behind ---
            if u >= 1:
                emit_reduce_upto(9 * (u - 1) + RED_CUM[h])

    emit_reduce_upto(N_GROUPS)


def build():
    nc = bacc.Bacc("TRN2", target_bir_lowering=False, debug=False)
    v_d = nc.dram_tensor("vwin", [N_UNITS, 45 * VROW], BF16,
                         kind="ExternalInput").ap()
    wc_d = nc.dram_tensor("wconsts", [125, 372], BF16,
                          kind="ExternalInput").ap()
    y_d = nc.dram_tensor("y", [B_LOC, NPIX], F32, kind="ExternalOutput").ap()

    with tile.TileContext(nc) as tc:
        with ExitStack() as ctx:
            _emit(ctx, tc, v_d, wc_d, y_d)
    nc.compile()
    return nc


def make_consts(W, b):
    import ml_dtypes

    W = np.asarray(W, dtype=np.float32)
    b = np.asarray(b, dtype=np.float32)
    # tap index t = 5a+bb (a=row-shift, bb=col-shift); gate channel at
    # slot q=(aq,bq) is conv output channel c = 5*bq+aq
    perm = np.array([5 * (q % 5) + q // 5 for q in range(25)])
    W2t = W[:, 0].reshape(25, 25)          # W2t[c, t] = W[c,0,a,bb]
    wsmall = W2t[perm].T                   # wsmall[t, q] = W2t[perm[q], t]
    wc = np.zeros((125, 372), dtype=np.float32)
    for s in range(5):
        wc[25 * s : 25 * s + 25, 25 * s : 25 * s + 25] = wsmall  # wblk
        wc[25 * s : 25 * s + 25, 125 + 120 + s] = 1.0            # ones
    wc[:, 370] = np.tile(b[perm], 5)                             # bias
    return wc.astype(ml_dtypes.bfloat16)


def make_windows(x):
    """Dense im2col in unit order: vwin[core][u, s, t, g', pix] =
    pad4(x)[img=45u+5g'+s, 2+a+y, 2+bb+x], bf16; image slots >= 256 zero."""
    import ml_dtypes

    x = np.asarray(x, dtype=np.float32).reshape(B_FULL, 28, 28)
    xp4 = np.pad(x, ((0, 0), (4, 4), (4, 4)))
    win = np.lib.stride_tricks.sliding_window_view(xp4, (28, 28), axis=(1, 2))
    win = win[:, 2:7, 2:7]                 # [B, 5, 5, 28, 28]
    win = win.reshape(N_CORES, B_LOC, 25, NPIX).astype(ml_dtypes.bfloat16)
    vw = np.zeros((N_CORES, 45 * N_UNITS, 25, NPIX),
                  dtype=ml_dtypes.bfloat16)
    vw[:, :B_LOC] = win
    # [c, (u g' s), t, pix] -> [c, u, s, t, g', pix]
    vw = vw.reshape(N_CORES, N_UNITS, 9, 5, 25, NPIX)
    vw = vw.transpose(0, 1, 3, 4, 2, 5)
    return np.ascontiguousarray(
        vw.reshape(N_CORES, N_UNITS, 45 * VROW)
    )


_NC_CACHE = None


def get_nc():
    global _NC_CACHE
    if _NC_CACHE is None:
        _NC_CACHE = build()
    return _NC_CACHE


def run(x, W, b, **spmd_kwargs):
    wc = make_consts(W, b)
    vw = make_windows(x)
    in_maps = [{"vwin": vw[c], "wconsts": wc} for c in range(N_CORES)]
    nc = get_nc()
    res = bass_utils.run_bass_kernel_spmd(
        nc, in_maps, list(range(N_CORES)), **spmd_kwargs
    )
    y = np.concatenate([res.results[c]["y"] for c in range(N_CORES)], axis=0)
    return y.reshape(B_FULL, 1, 28, 28), res


def kernel(x, W, b):
    y, _ = run(x, W, b)
    return y.astype(np.float32)
